# revision 5
# baseline (speedup 1.0000x reference)
"""Trainium2 Bass kernel for RealVirtualAttention (masked segment-mean pool + HAN
semantic attention), SPMD across 8 NeuronCores.  v5: v4 + halved chunk DMAs
(compute starts after the first half lands) + bf16 final-combine datapath. v4: v3 + rank-matched block
slots (variable per-slot tile counts), deeper prefetch, HWDGE collective DMAs.

Strategy (data-parallel over graphs):
  - 4096 graphs -> 128 blocks of GB=32 graphs; core i owns blocks 16i..16i+15
    (nodes sharded at graph boundaries, batch is sorted).
  - Each core sorts its 16 blocks by node count (descending) into SLOTS; slot
    j's tile count T_j = max over cores of the j-th largest block, so padding
    is near the per-core ragged minimum while staying SPMD-uniform. The host
    un-permutes slot-major output rows at the end.
  - Node features bf16 on host, packed [128, T_j*150] per slot; one DMA per
    slot. Block-local col ids (col = (batch % GB) + GB*(z==VIRT), -1 padding)
    ride in a packed f32 const blob (single DMA).
  - Per 128-node tile, a [128, 64] bf16 one-hot selector is built on DVE
    (is_equal vs iota) and matmul-accumulated selT.T @ X into a [64, 150]
    PSUM bank -> per-slot masked segment sums (rows: 32 real | 32 virtual).
  - Slot epilogue (ACT scale by 1/count -> means, transposes, W1/tanh/q HAN
    head) is emitted software-pipelined into the next slot's matmul stream.
  - Scores all-reduced (8 bytes) across cores for the global softmax beta;
    final combine = 5 wide bsel.T @ means matmuls; single contiguous output
    DMA of the core's 512-graph slice.
"""

import numpy as np
import ml_dtypes

import concourse.bacc as bacc
import concourse.bass as bass
import concourse.tile as tile
import concourse.mybir as mybir
from concourse.bass_utils import run_bass_kernel_spmd

F32 = mybir.dt.float32
BF16 = mybir.dt.bfloat16
NPBF16 = ml_dtypes.bfloat16
N_CORES = 8
B = 4096          # graphs
D = 150           # feature dim
A = 128           # attention hidden dim
GB = 32           # graphs per block
SW = 2 * GB       # selector width (real|virtual slots per block)
NBLK = 16         # blocks per core
VIRTUAL_Z = 100

_PROGRAM_CACHE: dict = {}
LAST_RESULTS = None  # BassKernelResults of the most recent run (for test.py)
LAST_NC = None       # compiled program of the most recent run (for test.py)
LAST_IN_MAPS = None  # per-core input maps of the most recent run (for test.py)


def _program_params(batch):
    """Per-slot tile counts: slot j holds each core's j-th largest block."""
    cuts = np.searchsorted(batch, np.arange(0, B + 1, GB))
    nb = np.diff(cuts).reshape(N_CORES, NBLK)
    s = -np.sort(-nb, axis=1)
    Tj = np.maximum(np.ceil(s.max(axis=0) / 128.0).astype(int), 1)
    return tuple(int(t) for t in Tj), 0


def _blob_offsets(T_list):
    """Column offsets of the packed f32 const blob [128, CBLOB]."""
    TT = sum(T_list)
    off = {}
    c = 0
    for name, w in [("ident", 128), ("col", TT), ("scales", NBLK),
                    ("e0", GB), ("e1", GB), ("w1a", A), ("w1b", A),
                    ("b1", 1), ("q", 1)]:
        off[name] = c
        c += w
    return off, c


def _build_program(T_list, _unused: int = 0):
    key = ("v6", tuple(T_list))
    if key in _PROGRAM_CACHE:
        return _PROGRAM_CACHE[key]

    TT = sum(T_list)
    TOTF = TT * D
    OFF, CBLOB = _blob_offsets(T_list)
    offx = np.concatenate([[0], np.cumsum([t * D for t in T_list])])
    offc = np.concatenate([[0], np.cumsum(T_list)])

    nc = bacc.Bacc("TRN2", target_bir_lowering=False, debug=False,
                   num_devices=N_CORES)
    xdat = nc.declare_dram_parameter("xdat", [128, TOTF], BF16, isOutput=False)
    blobp = nc.declare_dram_parameter("blob", [128, CBLOB], F32, isOutput=False)
    iotap = nc.declare_dram_parameter("iota", [128, SW], BF16, isOutput=False)
    resp = nc.declare_dram_parameter("res", [GB, NBLK, D], F32, isOutput=True)

    with tile.TileContext(nc) as tc:
        with tc.tile_pool(name="const", bufs=1) as cpool, \
             tc.tile_pool(name="chunks", bufs=8) as chpool, \
             tc.tile_pool(name="oh", bufs=24) as ohpool, \
             tc.tile_pool(name="small", bufs=1) as spool, \
             tc.tile_pool(name="xt", bufs=2) as xtpool, \
             tc.tile_pool(name="pm", bufs=2, space="PSUM") as pm, \
             tc.tile_pool(name="ptp", bufs=1, space="PSUM") as ptp, \
             tc.tile_pool(name="ph", bufs=1, space="PSUM") as ph, \
             tc.tile_pool(name="ps", bufs=1, space="PSUM") as ps, \
             tc.tile_pool(name="pbb", bufs=1, space="PSUM") as pbbp, \
             tc.tile_pool(name="pout", bufs=2, space="PSUM") as pout, \
             tc.tile_pool(name="dram", bufs=1, space="DRAM") as dpool:

            # --- constants (one blob DMA + iota) ---
            blob_t = cpool.tile([128, CBLOB], F32, tag="blob")
            nc.scalar.dma_start(blob_t[:], blobp[:])
            iota_t = cpool.tile([128, SW], BF16, tag="iota")
            nc.scalar.dma_start(iota_t[:], iotap[:])

            def bs(name, w, p=128):
                c = OFF[name]
                return blob_t[0:p, c:c + w]

            ident64 = blob_t[0:SW, 0:SW]
            col_t = bs("col", TT)
            scales_t = bs("scales", NBLK)       # rows 0:SW used
            e0_t = blob_t[0:SW, OFF["e0"]:OFF["e0"] + GB]
            e1_t = blob_t[0:SW, OFF["e1"]:OFF["e1"] + GB]
            w1a_t = bs("w1a", A)
            w1b_t = blob_t[0:D - 128, OFF["w1b"]:OFF["w1b"] + A]
            b1_t = bs("b1", 1)
            q_t = bs("q", 1)

            means_all = cpool.tile([SW, NBLK * D], F32, tag="means")
            means_bf = cpool.tile([SW, NBLK * D], BF16, tag="meansbf")
            scores_all = cpool.tile([1, 2 * NBLK * GB], F32, tag="sall")
            osb_all = cpool.tile([GB, NBLK * D], F32, tag="osb")

            eq = mybir.AluOpType.is_equal
            mult = mybir.AluOpType.mult

            psum_tiles = [None] * NBLK

            def epilogue(j):
                """means scale + HAN head for slot j (ACT/PE only; no DVE)."""
                msl = means_all[:, j * D:(j + 1) * D]
                nc.scalar.mul(msl, psum_tiles[j][:],
                              scales_t[0:SW, j:j + 1])
                nc.scalar.copy(means_bf[:, j * D:(j + 1) * D], msl)
                tp = ptp.tile([128, 128], F32, tag="tp")
                nc.tensor.transpose(tp[:, 0:SW], msl[:, 0:128], ident64)
                nc.tensor.transpose(tp[0:D - 128, SW:2 * SW], msl[:, 128:D],
                                    ident64)
                xt = xtpool.tile([128, 128], F32, tag="xt")
                nc.scalar.copy(xt[:], tp[:])
                ph_t = ph.tile([128, SW], F32, tag="h")
                nc.tensor.matmul(ph_t[:], w1a_t, xt[:, 0:SW],
                                 start=True, stop=False)
                nc.tensor.matmul(ph_t[:], w1b_t, xt[0:D - 128, SW:2 * SW],
                                 start=False, stop=True)
                ht = xtpool.tile([128, SW], F32, tag="ht")
                nc.scalar.activation(ht[:], ph_t[:],
                                     mybir.ActivationFunctionType.Tanh,
                                     bias=b1_t)
                ps_t = ps.tile([1, SW], F32, tag="s")
                nc.tensor.matmul(ps_t[:], q_t, ht[:], start=True, stop=True)
                nc.scalar.copy(scores_all[:, j * GB:(j + 1) * GB],
                               ps_t[0:1, 0:GB])
                nc.scalar.copy(scores_all[:, (NBLK + j) * GB:(NBLK + j + 1) * GB],
                               ps_t[0:1, GB:SW])

            # --- main streaming loop: masked segment sums per slot ---
            for j in range(NBLK):
                Tj = T_list[j]
                psum_tiles[j] = pm.tile([SW, D], F32, tag="pmeans", name="pmeans")
                chunk = chpool.tile([128, Tj * D], BF16, tag="chunk")
                H = (Tj // 2) * D
                nc.sync.dma_start(chunk[:, 0:H], xdat[:, offx[j]:offx[j] + H])
                nc.sync.dma_start(chunk[:, H:Tj * D],
                                  xdat[:, offx[j] + H:offx[j + 1]])
                for t in range(Tj):
                    if t == 16 and j > 0:
                        epilogue(j - 1)
                    gt = offc[j] + t
                    oh = ohpool.tile([128, SW], BF16, tag="oh")
                    nc.vector.tensor_scalar(
                        out=oh[:], in0=iota_t[:],
                        scalar1=col_t[:, gt:gt + 1],
                        scalar2=None, op0=eq)
                    nc.tensor.matmul(psum_tiles[j][:], oh[:],
                                     chunk[:, t * D:(t + 1) * D],
                                     start=(t == 0), stop=(t == Tj - 1))
            epilogue(NBLK - 1)

            # --- global beta via 8-byte AllReduce + softmax ---
            s2 = spool.tile([1, 2], F32, tag="s2")
            nc.vector.reduce_sum(out=s2[0:1, 0:1],
                                 in_=scores_all[0:1, 0:NBLK * GB],
                                 axis=mybir.AxisListType.X)
            nc.vector.reduce_sum(out=s2[0:1, 1:2],
                                 in_=scores_all[0:1, NBLK * GB:2 * NBLK * GB],
                                 axis=mybir.AxisListType.X)
            cc_in = dpool.tile([1, 2], F32)
            cc_out = dpool.tile([1, 2], F32)
            nc.scalar.dma_start(cc_in[:], s2[:])
            nc.gpsimd.collective_compute(
                "AllReduce", mybir.AluOpType.add,
                replica_groups=[list(range(N_CORES))],
                ins=[cc_in.opt()], outs=[cc_out.opt()])
            sg = spool.tile([1, 2], F32, tag="sg")
            nc.scalar.dma_start(sg[:], cc_out[:])
            e = spool.tile([1, 2], F32, tag="e")
            nc.scalar.activation(e[:], sg[:], mybir.ActivationFunctionType.Exp,
                                 scale=1.0 / B)
            esum = spool.tile([1, 1], F32, tag="esum")
            nc.vector.reduce_sum(out=esum[:], in_=e[:], axis=mybir.AxisListType.X)
            erec = spool.tile([1, 1], F32, tag="erec")
            nc.vector.reciprocal(erec[:], esum[:])
            beta = spool.tile([1, 2], F32, tag="beta")
            nc.vector.tensor_scalar(out=beta[:], in0=e[:],
                                    scalar1=erec[0:1, 0:1], scalar2=None,
                                    op0=mult)
            ones_t = spool.tile([1, SW], F32, tag="ones")
            nc.vector.memset(ones_t[:], 1.0)
            pbb = pbbp.tile([SW, 2], F32, tag="bb")
            nc.tensor.matmul(pbb[:], ones_t[:], beta[:], start=True, stop=True)
            beta_bc = spool.tile([SW, 2], F32, tag="bbc")
            nc.scalar.copy(beta_bc[:], pbb[:])
            tmp1 = spool.tile([SW, GB], F32, tag="tmp1")
            nc.vector.tensor_scalar(out=tmp1[:], in0=e0_t,
                                    scalar1=beta_bc[:, 0:1], scalar2=None,
                                    op0=mult)
            tmp2 = spool.tile([SW, GB], F32, tag="tmp2")
            nc.vector.tensor_scalar(out=tmp2[:], in0=e1_t,
                                    scalar1=beta_bc[:, 1:2], scalar2=None,
                                    op0=mult)
            bsel = spool.tile([SW, GB], F32, tag="bsel")
            nc.vector.tensor_add(bsel[:], tmp1[:], tmp2[:])
            bsel_bf = spool.tile([SW, GB], BF16, tag="bselbf")
            nc.scalar.copy(bsel_bf[:], bsel[:])

            # --- final combine (5 wide matmuls) + single output DMA ---
            CW = 480
            NC5 = (NBLK * D + CW - 1) // CW
            for i in range(NC5):
                lo = i * CW
                hi = min(NBLK * D, lo + CW)
                po = pout.tile([GB, CW], F32, tag="po")
                nc.tensor.matmul(po[0:GB, 0:hi - lo], bsel_bf[:],
                                 means_bf[:, lo:hi], start=True, stop=True)
                nc.vector.tensor_copy(osb_all[:, lo:hi], po[0:GB, 0:hi - lo])
            nc.scalar.dma_start(resp[:], osb_all[:])

    nc.compile()
    _PROGRAM_CACHE[key] = nc
    return nc


def kernel(out, z, batch, W1, b1, q, num_graphs):
    global LAST_RESULTS, LAST_NC, LAST_IN_MAPS
    out = np.ascontiguousarray(np.asarray(out, dtype=np.float32))
    z = np.asarray(z).astype(np.int64)
    batch = np.asarray(batch).astype(np.int64)
    W1 = np.asarray(W1, dtype=np.float32)
    b1 = np.asarray(b1, dtype=np.float32)
    q = np.asarray(q, dtype=np.float32)
    assert int(num_graphs) == B
    assert out.shape[1] == D and W1.shape == (D, A)

    cuts = np.searchsorted(batch, np.arange(0, B + 1, GB))
    nb = np.diff(cuts).reshape(N_CORES, NBLK)
    T_list, _ = _program_params(batch)
    TT = sum(T_list)
    OFF, CBLOB = _blob_offsets(T_list)
    offx = np.concatenate([[0], np.cumsum([t * D for t in T_list])])
    offc = np.concatenate([[0], np.cumsum(T_list)])

    virt = (z == VIRTUAL_Z)
    keyv = 2 * batch + virt
    cnt = np.bincount(keyv, minlength=2 * B).reshape(B, 2).astype(np.float32)
    rcnt = 1.0 / np.maximum(cnt, 1.0)                       # [B, 2]

    xb = out.astype(NPBF16)
    colf = ((batch % GB) + GB * virt).astype(np.float32)    # block-local col id

    iota = np.tile(np.arange(SW, dtype=NPBF16), (128, 1))

    blob_common = np.zeros((128, CBLOB), dtype=np.float32)
    blob_common[:, OFF["ident"]:OFF["ident"] + 128] = np.eye(128)
    blob_common[0:GB, OFF["e0"]:OFF["e0"] + GB] = np.eye(GB)
    blob_common[GB:SW, OFF["e1"]:OFF["e1"] + GB] = np.eye(GB)
    blob_common[:, OFF["w1a"]:OFF["w1a"] + A] = W1[:128]
    blob_common[0:D - 128, OFF["w1b"]:OFF["w1b"] + A] = W1[128:]
    blob_common[:, OFF["b1"]] = b1
    blob_common[:, OFF["q"]] = q.reshape(A)

    in_maps = []
    orders = []
    for core in range(N_CORES):
        order = np.argsort(-nb[core], kind="stable")        # slot j <- block order[j]
        orders.append(order)
        arr = np.zeros((TT * 128, D), dtype=NPBF16)
        colv = np.full(TT * 128, -1.0, dtype=np.float32)
        blob = blob_common.copy()
        g0 = core * NBLK * GB
        for j in range(NBLK):
            k = NBLK * core + int(order[j])
            lo, hi = int(cuts[k]), int(cuts[k + 1])
            nbk = hi - lo
            base = offc[j] * 128
            arr[base:base + nbk] = xb[lo:hi]
            colv[base:base + nbk] = colf[lo:hi]
            gids = g0 + int(order[j]) * GB + np.arange(GB)
            blob[0:GB, OFF["scales"] + j] = rcnt[gids, 0]
            blob[GB:SW, OFF["scales"] + j] = rcnt[gids, 1]
        # per-slot tile-major layout: [128, sum_j T_j*D]
        xarr = np.empty((128, TT * D), dtype=NPBF16)
        for j in range(NBLK):
            seg = arr[offc[j] * 128:offc[j + 1] * 128]      # [T_j*128, D]
            Tj = T_list[j]
            xarr[:, offx[j]:offx[j + 1]] = (
                seg.reshape(Tj, 128, D).transpose(1, 0, 2).reshape(128, Tj * D))
        blob[:, OFF["col"]:OFF["col"] + TT] = colv.reshape(TT, 128).T
        in_maps.append({"xdat": xarr, "blob": blob, "iota": iota})

    nc = _build_program(T_list)
    LAST_NC, LAST_IN_MAPS = nc, in_maps
    res = run_bass_kernel_spmd(nc, in_maps, core_ids=list(range(N_CORES)))
    LAST_RESULTS = res
    parts = []
    for i in range(N_CORES):
        slots = res.results[i]["res"].transpose(1, 0, 2)    # [NBLK slots, GB, D]
        blocks = np.empty_like(slots)
        blocks[orders[i]] = slots                           # un-permute
        parts.append(blocks.reshape(NBLK * GB, D))
    return np.ascontiguousarray(np.concatenate(parts, axis=0), dtype=np.float32)
